# revision 1
# baseline (speedup 1.0000x reference)
"""Trainium2 Bass kernel for nn_Attention_48541720379807.

Multi-head attention (N=8 heads, H=128) with per-head K/Q projections,
softmax over projected keys, attention applied to projected keys, head
concat, and an output Linear.  B=8, L=2048, E=1024.

Sharding: pure data parallel - batch element b -> NeuronCore b.  Each core
computes its full batch slice including the output projection; the host
slices inputs and stacks outputs.  No collectives.

Per-core pipeline (PE matmuls; layouts avoid any on-device transpose of
the big input tensors - host supplies kT/qT/proj_w.T):
  A:  kxT[n] (H,L) = w_kx[n].T @ k.T   (lhsT = w slices, rhs = kT chunks)
      qxT[n] likewise; both spilled to DRAM scratch to bound SBUF.
  B:  per head, per 512-wide q block:
        scoreT[kt] (128,512) = kxT[:,kt-block].T @ qxT[:,qblk]   (PE)
        expT[kt]   = exp(score * 1/sqrt(H))            (ACT, scale fused)
        outT (H,512) += kx_nat[kc].T @ expT[kc]        (PE, accum over k)
        denom (1,512) += ones.T @ expT[kc]             (PE, interleaved)
        out_norm[:,qblk] = outT * bcast(1/denom)       (GPSIMD + DVE)
      kx_nat (k-major copy of kxT) from 16 PE transposes per head.
  C:  y (L,E) = sum_c out_norm_c[qt].T @ pwT_c + b     (PE, accum over c)

dtype mode: "f32r" (tf32 mantissa, full PE rate) or "f32" (exact fp32,
1/4 PE rate).  f32r matmul operands must be tf32-rounded by their
producer: host inputs are pre-rounded on the host; on-device
intermediates are written into f32r tiles (the copy/activation rounds).
"""

import math

import numpy as np

B, L, E, N, H = 8, 2048, 1024, 8, 128
NCORES = 8
QBLK = 512          # q block width in phase B
KCH = L // 128      # 16 k chunks / k tiles
ECH = E // 128      # 8 e chunks
SCALE = 1.0 / math.sqrt(H)

MODE = "f32"        # "f32" (exact) | "f32r" (tf32-fast)

_CACHE = {}
_last_in_maps = None


def _round_tf32(x):
    u = np.ascontiguousarray(x, dtype=np.float32).view(np.uint32)
    add = ((u >> 13) & np.uint32(1)) + np.uint32(0x0FFF)
    return ((u + add) & np.uint32(0xFFFFE000)).view(np.float32)


def _build(mode):
    from concourse import bacc
    import concourse.mybir as mybir
    from concourse.tile import TileContext
    from concourse.masks import make_identity

    f32 = mybir.dt.float32
    mdt = mybir.dt.float32r if mode == "f32r" else f32

    nc = bacc.Bacc("TRN2", target_bir_lowering=False, debug=False,
                   num_devices=NCORES)

    kT_d = nc.dram_tensor("kT", [E, L], mdt, kind="ExternalInput")
    qT_d = nc.dram_tensor("qT", [E, L], mdt, kind="ExternalInput")
    wk_d = nc.dram_tensor("wk", [E, N * H], mdt, kind="ExternalInput")
    wq_d = nc.dram_tensor("wq", [E, N * H], mdt, kind="ExternalInput")
    pwT_d = nc.dram_tensor("pwT", [N * H, E], mdt, kind="ExternalInput")
    pb_d = nc.dram_tensor("pb", [1, E], f32, kind="ExternalInput")
    y_d = nc.dram_tensor("y", [L, E], f32, kind="ExternalOutput")
    qxT_sc = nc.dram_tensor("qxT_sc", [N * H, L], mdt)
    kxT_sc = nc.dram_tensor("kxT_sc", [N * H, L], mdt)
    on_sc = nc.dram_tensor("on_sc", [N * H, L], mdt)

    with TileContext(nc) as tc:
        with (
            tc.tile_pool(name="const", bufs=1) as const,
            tc.tile_pool(name="wsl", bufs=3 if mode == "f32r" else 2) as wsl,
            tc.tile_pool(name="ktp", bufs=1) as ktp,       # 8x (128,1024) kT/qT half-chunks
            tc.tile_pool(name="evp", bufs=2) as evp,       # (128,1024) phase-A evict
            tc.tile_pool(name="kxth", bufs=2) as kxth,     # per-head kxT (128,2048)
            tc.tile_pool(name="onh", bufs=2) as onh,       # per-head out_norm (128,2048)
            tc.tile_pool(name="qxh", bufs=2) as qxh,       # per-head qxT (128,2048)
            tc.tile_pool(name="kxn", bufs=2 if mode == "f32r" else 1) as kxn,       # per-head kx_nat (128,2048)
            tc.tile_pool(name="expp", bufs=10 if mode == "f32r" else 7) as expp,
            tc.tile_pool(name="erp", bufs=2) as erp,       # f32-mode denom operands
            tc.tile_pool(name="small", bufs=1) as small,
            tc.tile_pool(name="psA", bufs=2, space="PSUM") as psA,
            tc.tile_pool(name="psO", bufs=2, space="PSUM") as psO,
            tc.tile_pool(name="psD", bufs=2, space="PSUM") as psD,
        ):
            ident_f = const.tile([128, 128], f32)
            make_identity(nc, ident_f)
            ident = const.tile([128, 128], mdt)
            nc.vector.tensor_copy(ident[:], ident_f[:])
            ones_f = const.tile([128, 1], f32)
            nc.any.memset(ones_f[:], 1.0)
            ones = const.tile([128, 1], mybir.dt.float32r)
            nc.vector.tensor_copy(ones[:], ones_f[:])
            pb_sb = const.tile([1, E], f32)
            nc.sync.dma_start(out=pb_sb[:], in_=pb_d[:])
            pb_bc = const.tile([128, E], f32)
            nc.gpsimd.partition_broadcast(pb_bc[:], pb_sb[:])

            # ---------------- Phase A ----------------
            def phase_a(src_d, w_d, dst_sc, sweep0):
                for lh in range(2):          # l halves of 1024
                    sweep = sweep0 + lh
                    ls = slice(lh * 1024, (lh + 1) * 1024)
                    src_tiles = []
                    for ec in range(ECH):
                        st = ktp.tile([128, 1024], mdt,
                                      tag=f"kt{(sweep * ECH + ec) % 11}")
                        for hh in range(2):
                            nc.sync.dma_start(
                                out=st[:, hh * 512:(hh + 1) * 512],
                                in_=src_d[ec * 128:(ec + 1) * 128,
                                          lh * 1024 + hh * 512:
                                          lh * 1024 + (hh + 1) * 512])
                        src_tiles.append(st)
                    for n in range(N):
                        wt = wsl.tile([128, ECH * H], mdt, tag="wcat")
                        nc.sync.dma_start(
                            out=wt[:].rearrange("p (c h) -> p c h", c=ECH),
                            in_=w_d[:, n * H:(n + 1) * H].rearrange(
                                "(c p) h -> p c h", p=128))
                        ev = evp.tile([128, 1024], mdt, tag="ev")
                        ps = psA.tile([128, 1024], f32, tag="psA")
                        for ec in range(ECH):
                            for lb in range(2):
                                nc.tensor.matmul(
                                    ps[:, lb * 512:(lb + 1) * 512],
                                    wt[:, ec * H:(ec + 1) * H],
                                    src_tiles[ec][:, lb * 512:(lb + 1) * 512],
                                    start=(ec == 0), stop=(ec == ECH - 1))
                        nc.vector.tensor_copy(ev[:], ps[:])
                        nc.gpsimd.dma_start(
                            out=dst_sc[n * H:(n + 1) * H, ls], in_=ev[:])

            with nc.named_scope("A_q"):
                phase_a(qT_d, wq_d, qxT_sc, 0)
            with nc.named_scope("A_k"):
                phase_a(kT_d, wk_d, kxT_sc, 2)

            # pwT prefetch (ktp tags are free after phase A)
            pw_tiles = []
            for c in range(N):
                pwt = ktp.tile([128, E], mdt, tag=f"kt{c}")
                nc.sync.dma_start(out=pwt[:], in_=pwT_d[c * 128:(c + 1) * 128, :])
                pw_tiles.append(pwt)

            # ---------------- Phase B ----------------
            for n in range(N):
              with nc.named_scope(f"B{n}"):
                  kxT = kxth.tile([128, L], mdt, tag="kxt")
                  nc.sync.dma_start(out=kxT[:], in_=kxT_sc[n * H:(n + 1) * H, :])
                  qxT = qxh.tile([128, L], mdt, tag="qh")
                  nc.sync.dma_start(out=qxT[:], in_=qxT_sc[n * H:(n + 1) * H, :])

                  # kx_nat: (k in chunk = partition, [chunk, h] on free)
                  kx_nat = kxn.tile([128, KCH * H], mdt, tag="kxn")
                  for grp in range(KCH // 4):
                      pt = psD.tile([128, 512], mdt, tag="trp")
                      for j in range(4):
                          kc = grp * 4 + j
                          nc.tensor.transpose(
                              pt[:, j * 128:(j + 1) * 128],
                              kxT[:, kc * 128:(kc + 1) * 128], ident[:])
                      nc.vector.tensor_copy(
                          kx_nat[:, grp * 512:(grp + 1) * 512], pt[:])
                  if mode == "f32":
                      # f32r hi/lo split of kx_nat: AV runs as 3 f32r passes
                      kxn_hi = kxn.tile([128, KCH * H],
                                        mybir.dt.float32r, tag="kxnh")
                      nc.vector.tensor_copy(kxn_hi[:], kx_nat[:])
                      kxn_lo = kxn.tile([128, KCH * H],
                                        mybir.dt.float32r, tag="kxnl")
                      nc.vector.tensor_sub(kxn_lo[:], kx_nat[:], kxn_hi[:])

                  on = onh.tile([128, L], mdt, tag="on")
                  for qb in range(L // QBLK):
                      qs = slice(qb * QBLK, (qb + 1) * QBLK)
                      pair_tiles = []
                      for p in range(KCH // 2):
                          ps_s = psA.tile([128, 2 * QBLK], f32, tag="psA")
                          for j in range(2):
                              kt = 2 * p + j
                              nc.tensor.matmul(
                                  ps_s[:, j * QBLK:(j + 1) * QBLK],
                                  kxT[:, kt * 128:(kt + 1) * 128],
                                  qxT[:, qs], start=True, stop=True)
                          et = expp.tile([128, 2 * QBLK], mdt, tag="expt")
                          nc.scalar.activation(
                              et[:], ps_s[:], mybir.ActivationFunctionType.Exp,
                              scale=SCALE)
                          pair_tiles.append(et)
                      if mode == "f32r":
                          den_tiles = pair_tiles
                      else:
                          den_tiles = []
                          for p in range(KCH // 2):
                              er = erp.tile([128, 2 * QBLK],
                                            mybir.dt.float32r, tag="er")
                              nc.vector.tensor_copy(er[:], pair_tiles[p][:])
                              er2 = erp.tile([128, 2 * QBLK],
                                             mybir.dt.float32r, tag="er2")
                              nc.vector.tensor_sub(
                                  er2[:], pair_tiles[p][:], er[:])
                              den_tiles.append((er, er2))
                      ps_o = psO.tile([128, QBLK], f32, tag="psO")
                      ps_d = psD.tile([1, QBLK], f32, tag="trp")
                      for kc in range(KCH):
                          sl = slice((kc % 2) * QBLK, (kc % 2 + 1) * QBLK)
                          esl = pair_tiles[kc // 2][:, sl]
                          hs = slice(kc * H, (kc + 1) * H)
                          if mode == "f32r":
                              nc.tensor.matmul(
                                  ps_o[:], kx_nat[:, hs], esl,
                                  start=(kc == 0), stop=(kc == KCH - 1))
                              nc.tensor.matmul(
                                  ps_d[:], ones[:], esl,
                                  start=(kc == 0), stop=(kc == KCH - 1))
                          else:
                              # out += kxh.T@eh + kxl.T@eh + kxh.T@el
                              er, er2 = den_tiles[kc // 2]
                              nc.tensor.matmul(
                                  ps_o[:], kxn_hi[:, hs], er[:, sl],
                                  start=(kc == 0), stop=False)
                              nc.tensor.matmul(
                                  ps_o[:], kxn_lo[:, hs], er[:, sl],
                                  start=False, stop=False)
                              nc.tensor.matmul(
                                  ps_o[:], kxn_hi[:, hs], er2[:, sl],
                                  start=False, stop=(kc == KCH - 1))
                              nc.tensor.matmul(
                                  ps_d[:], ones[:], er[:, sl],
                                  start=(kc == 0), stop=False)
                              nc.tensor.matmul(
                                  ps_d[:], ones[:], er2[:, sl],
                                  start=False, stop=(kc == KCH - 1))
                      d_sb = small.tile([1, QBLK], f32, tag="dsb")
                      nc.vector.tensor_copy(d_sb[:], ps_d[:])
                      d_bc = small.tile([128, QBLK], f32, tag="dbc")
                      nc.gpsimd.partition_broadcast(d_bc[:], d_sb[:])
                      d_rc = small.tile([128, QBLK], f32, tag="drc")
                      nc.vector.reciprocal(d_rc[:], d_bc[:])
                      nc.vector.tensor_mul(on[:, qs], ps_o[:], d_rc[:])
                  if n == N - 1:
                      on_last = on
                  else:
                      nc.gpsimd.dma_start(
                          out=on_sc[n * H:(n + 1) * H, :], in_=on[:])

            # ---------------- Phase C ----------------
            scope_c = nc.named_scope("C")
            scope_c.__enter__()
            for qt in range(L // 128):
                oncat = wsl.tile([128, (N - 1) * 128], mdt, tag="oncat")
                nc.sync.dma_start(
                    out=oncat[:].rearrange("p (c j) -> p c j", c=N - 1),
                    in_=on_sc[0:(N - 1) * H, qt * 128:(qt + 1) * 128].rearrange(
                        "(c p) j -> p c j", p=128))
                on_tiles = [oncat[:, c * 128:(c + 1) * 128]
                            for c in range(N - 1)]
                on_tiles.append(on_last[:, qt * 128:(qt + 1) * 128])
                y_sb = evp.tile([128, E], f32, tag="ysb")
                ps_y = psA.tile([128, 1024], f32, tag="psA")
                for c in range(N):
                    for eb in range(E // 512):
                        nc.tensor.matmul(
                            ps_y[:, eb * 512:(eb + 1) * 512],
                            on_tiles[c],
                            pw_tiles[c][:, eb * 512:(eb + 1) * 512],
                            start=(c == 0), stop=(c == N - 1))
                nc.vector.tensor_add(y_sb[:], ps_y[:], pb_bc[:])
                nc.sync.dma_start(out=y_d[qt * 128:(qt + 1) * 128, :], in_=y_sb[:])
            scope_c.__exit__(None, None, None)

    nc.compile()
    return nc


def _get_program(mode=MODE):
    if mode not in _CACHE:
        _CACHE[mode] = _build(mode)
    return _CACHE[mode]


def kernel(k, q, w_kx, w_qx, proj_w, proj_b, mode=MODE):
    from concourse.bass_utils import run_bass_kernel_spmd

    k = np.asarray(k, dtype=np.float32)
    q = np.asarray(q, dtype=np.float32)
    w_kx = np.asarray(w_kx, dtype=np.float32)
    w_qx = np.asarray(w_qx, dtype=np.float32)
    proj_w = np.asarray(proj_w, dtype=np.float32)
    proj_b = np.asarray(proj_b, dtype=np.float32)

    rnd = _round_tf32 if mode == "f32r" else (
        lambda x: np.ascontiguousarray(x, dtype=np.float32))
    wk = rnd(w_kx.transpose(1, 0, 2).reshape(E, N * H))   # (e, n*h)
    wq = rnd(w_qx.transpose(1, 0, 2).reshape(E, N * H))
    pwT = rnd(proj_w.T)
    pb = np.ascontiguousarray(proj_b.reshape(1, E), dtype=np.float32)

    in_maps = []
    for b in range(NCORES):
        in_maps.append({
            "kT": rnd(k[b].T),
            "qT": rnd(q[b].T),
            "wk": wk,
            "wq": wq,
            "pwT": pwT,
            "pb": pb,
        })

    global _last_in_maps
    _last_in_maps = in_maps
    nc = _get_program(mode)
    res = run_bass_kernel_spmd(nc, in_maps, list(range(NCORES)))
    out = np.stack([res.results[b]["y"] for b in range(NCORES)], axis=0)
    return out.astype(np.float32)



# revision 6
# speedup vs baseline: 1.0957x; 1.0957x over previous
"""Trainium2 Bass kernel for nn_Attention_48541720379807.

Multi-head attention (N=8 heads, H=128) with per-head K/Q projections,
softmax over projected keys, attention applied to projected keys, head
concat, and an output Linear.  B=8, L=2048, E=1024.

Sharding: pure data parallel - batch element b -> NeuronCore b.  Each core
computes its full batch slice including the output projection; the host
slices inputs and stacks outputs.  No collectives.

Per-core pipeline (PE matmuls; layouts avoid any on-device transpose of
the big input tensors - host supplies kT/qT/proj_w.T):
  A:  kxT[n] (H,L) = w_kx[n].T @ k.T   (lhsT = w slices, rhs = kT chunks)
      qxT[n] likewise; both spilled to DRAM scratch to bound SBUF.
  B:  per head, per 512-wide q block:
        scoreT[kt] (128,512) = kxT[:,kt-block].T @ qxT[:,qblk]   (PE)
        expT[kt]   = exp(score * 1/sqrt(H))            (ACT, scale fused)
        fold[p]    = expT[2p] + expT[2p+1]             (DVE/Pool, halves
                                                        the denom matmuls)
        outT (H,512) += kx_nat[kc].T @ expT[kc]        (PE, accum over k)
        denom (128,512) += ones128.T @ fold[p]         (PE, pre-broadcast;
                                                        deferred into the
                                                        NEXT qb's stream so
                                                        the ACT-lagged tail
                                                        never stalls PE)
        out_norm[:,qblk] = outT * recip_fast(denom)    (DVE)
      kx_nat (k-major copy of kxT) from 16 PE transposes per head.
      kxT/qxT DMA loads are prefetched one head ahead; pw tiles stream in
      one per head.
  C:  y (L,E) = sum_c out_norm_c[qt].T @ pwT_c + b     (PE, accum over c)

dtype mode: "f32r" (tf32 mantissa, full PE rate) or "f32" (exact fp32,
1/4 PE rate).  f32r matmul operands must be tf32-rounded by their
producer: host inputs are pre-rounded on the host; on-device
intermediates are written into f32r tiles (the copy/activation rounds).
"""

import math

import numpy as np

B, L, E, N, H = 8, 2048, 1024, 8, 128
NCORES = 8
QBLK = 512          # q block width in phase B
KCH = L // 128      # 16 k chunks / k tiles
ECH = E // 128      # 8 e chunks
SCALE = 1.0 / math.sqrt(H)

MODE = "f32r"       # "f32" (exact) | "f32r" (tf32-fast)

_CACHE = {}
_last_in_maps = None


def _round_tf32(x):
    u = np.ascontiguousarray(x, dtype=np.float32).view(np.uint32)
    add = ((u >> 13) & np.uint32(1)) + np.uint32(0x0FFF)
    return ((u + add) & np.uint32(0xFFFFE000)).view(np.float32)


def _build(mode):
    from concourse import bacc
    import concourse.mybir as mybir
    from concourse.tile import TileContext
    from concourse.masks import make_identity

    f32 = mybir.dt.float32
    mdt = mybir.dt.float32r if mode == "f32r" else f32

    nc = bacc.Bacc("TRN2", target_bir_lowering=False, debug=False,
                   num_devices=NCORES)

    kT_d = nc.dram_tensor("kT", [E, L], mdt, kind="ExternalInput")
    qT_d = nc.dram_tensor("qT", [E, L], mdt, kind="ExternalInput")
    wk_d = nc.dram_tensor("wk", [E, N * H], mdt, kind="ExternalInput")
    wq_d = nc.dram_tensor("wq", [E, N * H], mdt, kind="ExternalInput")
    pwT_d = nc.dram_tensor("pwT", [N * H, E], mdt, kind="ExternalInput")
    pb_d = nc.dram_tensor("pb", [1, E], f32, kind="ExternalInput")
    y_d = nc.dram_tensor("y", [L, E], f32, kind="ExternalOutput")
    qxT_sc = nc.dram_tensor("qxT_sc", [N * H, L], mdt)
    kxT_sc = nc.dram_tensor("kxT_sc", [N * H, L], mdt)
    on_sc = nc.dram_tensor("on_sc", [N * H, L], mdt)

    with TileContext(nc) as tc:
        with (
            tc.tile_pool(name="const", bufs=1) as const,
            tc.tile_pool(name="wsl", bufs=3 if mode == "f32r" else 2) as wsl,
            tc.tile_pool(name="ktp", bufs=1) as ktp,       # 8x (128,1024) kT/qT half-chunks
            tc.tile_pool(name="evp", bufs=2) as evp,       # (128,1024) phase-A evict
            tc.tile_pool(name="kxth", bufs=2) as kxth,     # per-head kxT (128,2048)
            tc.tile_pool(name="onh", bufs=2) as onh,       # per-head out_norm (128,2048)
            tc.tile_pool(name="qxh", bufs=2) as qxh,       # per-head qxT (128,2048)
            tc.tile_pool(name="kxn", bufs=1) as kxn,       # per-head kx_nat (128,2048)
            tc.tile_pool(name="expp", bufs=9 if mode == "f32r" else 7) as expp,
            tc.tile_pool(name="fldp", bufs=9) as fldp,     # f32r denom folds
            tc.tile_pool(name="erp", bufs=2) as erp,       # f32-mode denom operands
            tc.tile_pool(name="small", bufs=1) as small,
            tc.tile_pool(name="psA", bufs=2, space="PSUM") as psA,
            tc.tile_pool(name="psO", bufs=2, space="PSUM") as psO,
            tc.tile_pool(name="psD", bufs=2, space="PSUM") as psD,
        ):
            ident_f = const.tile([128, 128], f32)
            make_identity(nc, ident_f)
            ident = const.tile([128, 128], mdt)
            nc.vector.tensor_copy(ident[:], ident_f[:])
            ones_f = const.tile([128, 1], f32)
            nc.any.memset(ones_f[:], 1.0)
            ones = const.tile([128, 1], mybir.dt.float32r)
            nc.vector.tensor_copy(ones[:], ones_f[:])
            ones128_f = const.tile([128, 128], f32)
            nc.any.memset(ones128_f[:], 1.0)
            ones128 = const.tile([128, 128], mybir.dt.float32r)
            nc.vector.tensor_copy(ones128[:], ones128_f[:])
            pb_sb = const.tile([1, E], f32)
            nc.sync.dma_start(out=pb_sb[:], in_=pb_d[:])
            pb_bc = const.tile([128, E], f32)
            nc.gpsimd.partition_broadcast(pb_bc[:], pb_sb[:])

            # ---------------- Phase A ----------------
            def load_w(w_d, n):
                wt = wsl.tile([128, ECH * H], mdt, tag="wcat")
                nc.sync.dma_start(
                    out=wt[:].rearrange("p (c h) -> p c h", c=ECH),
                    in_=w_d[:, n * H:(n + 1) * H].rearrange(
                        "(c p) h -> p c h", p=128))
                return wt

            def phase_a(src_d, w_d, dst_sc, sweep0):
                for lh in range(2):          # l halves of 1024
                    sweep = sweep0 + lh
                    ls = slice(lh * 1024, (lh + 1) * 1024)
                    # first w tile BEFORE the 4MB of src DMAs so the first
                    # matmul isn't queued behind them
                    wt_next = load_w(w_d, 0)
                    src_tiles = []
                    for ec in range(ECH):
                        st = ktp.tile([128, 1024], mdt,
                                      tag=f"kt{(sweep * ECH + ec) % 11}")
                        for hh in range(2):
                            nc.sync.dma_start(
                                out=st[:, hh * 512:(hh + 1) * 512],
                                in_=src_d[ec * 128:(ec + 1) * 128,
                                          lh * 1024 + hh * 512:
                                          lh * 1024 + (hh + 1) * 512])
                        src_tiles.append(st)
                    for n in range(N):
                        wt = wt_next
                        if n + 1 < N:
                            wt_next = load_w(w_d, n + 1)
                        ev = evp.tile([128, 1024], mdt, tag="ev")
                        ps = psA.tile([128, 1024], f32, tag="psA")
                        for ec in range(ECH):
                            for lb in range(2):
                                nc.tensor.matmul(
                                    ps[:, lb * 512:(lb + 1) * 512],
                                    wt[:, ec * H:(ec + 1) * H],
                                    src_tiles[ec][:, lb * 512:(lb + 1) * 512],
                                    start=(ec == 0), stop=(ec == ECH - 1))
                        nc.vector.tensor_copy(ev[:], ps[:])
                        nc.gpsimd.dma_start(
                            out=dst_sc[n * H:(n + 1) * H, ls], in_=ev[:])

            with nc.named_scope("A_q"):
                phase_a(qT_d, wq_d, qxT_sc, 0)
            with nc.named_scope("A_k"):
                phase_a(kT_d, wk_d, kxT_sc, 2)

            # ---------------- Phase B ----------------
            def load_head(n):
                kt_ = kxth.tile([128, L], mdt, tag="kxt")
                nc.sync.dma_start(out=kt_[:],
                                  in_=kxT_sc[n * H:(n + 1) * H, :])
                qt_ = qxh.tile([128, L], mdt, tag="qh")
                nc.sync.dma_start(out=qt_[:],
                                  in_=qxT_sc[n * H:(n + 1) * H, :])
                return kt_, qt_

            # deferred work carried into the next qb's PE stream
            pending = []        # [(fold_tiles, ps_o, on, qs)]
            pending_ev = []     # [(on_tile, head)]

            def flush_denoms():
                if not pending:
                    return
                folds, ps_o_t, on_t, qs_ = pending.pop()
                ps_d = psD.tile([128, QBLK], f32, tag="psd")
                for p in range(KCH // 2):
                    nc.tensor.matmul(
                        ps_d[:], ones128[:], folds[p][:],
                        start=(p == 0), stop=(p == KCH // 2 - 1))
                d_rc = small.tile([128, QBLK], f32, tag="drc")
                nc.vector.reciprocal_approx_fast(d_rc[:], ps_d[:])
                nc.vector.tensor_mul(on_t[:, qs_], ps_o_t[:], d_rc[:])

            def flush_evict():
                if not pending_ev:
                    return
                on_t, n_ = pending_ev.pop()
                nc.gpsimd.dma_start(
                    out=on_sc[n_ * H:(n_ + 1) * H, :], in_=on_t[:])

            nxt = load_head(0)
            pw_tiles = []
            on_last = None
            for n in range(N):
              with nc.named_scope(f"B{n}"):
                  kxT, qxT = nxt
                  if n + 1 < N:
                      nxt = load_head(n + 1)
                  # stream one pw tile per head for phase C
                  pwt = ktp.tile([128, E], mdt, tag=f"kt{n}")
                  nc.sync.dma_start(out=pwt[:],
                                    in_=pwT_d[n * 128:(n + 1) * 128, :])
                  pw_tiles.append(pwt)

                  flush_denoms()
                  flush_evict()

                  # kx_nat: (k in chunk = partition, [chunk, h] on free)
                  kx_nat = kxn.tile([128, KCH * H], mdt, tag="kxn")
                  for grp in range(KCH // 4):
                      pt = psO.tile([128, 512], mdt, tag="psO")
                      for j in range(4):
                          kc = grp * 4 + j
                          nc.tensor.transpose(
                              pt[:, j * 128:(j + 1) * 128],
                              kxT[:, kc * 128:(kc + 1) * 128], ident[:])
                      nc.vector.tensor_copy(
                          kx_nat[:, grp * 512:(grp + 1) * 512], pt[:])
                  if mode == "f32":
                      # f32r hi/lo split of kx_nat: AV runs as 3 f32r passes
                      kxn_hi = kxn.tile([128, KCH * H],
                                        mybir.dt.float32r, tag="kxnh")
                      nc.vector.tensor_copy(kxn_hi[:], kx_nat[:])
                      kxn_lo = kxn.tile([128, KCH * H],
                                        mybir.dt.float32r, tag="kxnl")
                      nc.vector.tensor_sub(kxn_lo[:], kx_nat[:], kxn_hi[:])

                  on = onh.tile([128, L], mdt, tag="on")
                  for qb in range(L // QBLK):
                      qs = slice(qb * QBLK, (qb + 1) * QBLK)
                      if qb > 0:
                          flush_denoms()
                      pair_tiles = []
                      fold_tiles = []
                      for p in range(KCH // 2):
                          ps_s = psA.tile([128, 2 * QBLK], f32, tag="psA")
                          for j in range(2):
                              kt = 2 * p + j
                              nc.tensor.matmul(
                                  ps_s[:, j * QBLK:(j + 1) * QBLK],
                                  kxT[:, kt * 128:(kt + 1) * 128],
                                  qxT[:, qs], start=True, stop=True)
                          et = expp.tile([128, 2 * QBLK], mdt, tag="expt")
                          nc.scalar.activation(
                              et[:], ps_s[:], mybir.ActivationFunctionType.Exp,
                              scale=SCALE)
                          pair_tiles.append(et)
                          if mode == "f32r":
                              ft = fldp.tile([128, QBLK], mdt, tag="fld")
                              eng = nc.vector if p % 2 == 0 else nc.gpsimd
                              eng.tensor_add(ft[:], et[:, :QBLK], et[:, QBLK:])
                              fold_tiles.append(ft)
                      if mode == "f32r":
                          den_tiles = pair_tiles
                      else:
                          den_tiles = []
                          for p in range(KCH // 2):
                              er = erp.tile([128, 2 * QBLK],
                                            mybir.dt.float32r, tag="er")
                              nc.vector.tensor_copy(er[:], pair_tiles[p][:])
                              er2 = erp.tile([128, 2 * QBLK],
                                             mybir.dt.float32r, tag="er2")
                              nc.vector.tensor_sub(
                                  er2[:], pair_tiles[p][:], er[:])
                              den_tiles.append((er, er2))
                      ps_o = psO.tile([128, QBLK], f32, tag="psO")
                      if mode == "f32":
                          ps_d = psD.tile([128, QBLK], f32, tag="psd")
                      for kc in range(KCH):
                          sl = slice((kc % 2) * QBLK, (kc % 2 + 1) * QBLK)
                          esl = pair_tiles[kc // 2][:, sl]
                          hs = slice(kc * H, (kc + 1) * H)
                          if mode == "f32r":
                              nc.tensor.matmul(
                                  ps_o[:], kx_nat[:, hs], esl,
                                  start=(kc == 0), stop=(kc == KCH - 1))
                          else:
                              # out += kxh.T@eh + kxl.T@eh + kxh.T@el
                              er, er2 = den_tiles[kc // 2]
                              nc.tensor.matmul(
                                  ps_o[:], kxn_hi[:, hs], er[:, sl],
                                  start=(kc == 0), stop=False)
                              nc.tensor.matmul(
                                  ps_o[:], kxn_lo[:, hs], er[:, sl],
                                  start=False, stop=False)
                              nc.tensor.matmul(
                                  ps_o[:], kxn_hi[:, hs], er2[:, sl],
                                  start=False, stop=(kc == KCH - 1))
                              nc.tensor.matmul(
                                  ps_d[:], ones128[:], er[:, sl],
                                  start=(kc == 0), stop=False)
                              nc.tensor.matmul(
                                  ps_d[:], ones128[:], er2[:, sl],
                                  start=False, stop=(kc == KCH - 1))
                      if mode == "f32r":
                          pending.append((fold_tiles, ps_o, on, qs))
                      else:
                          d_rc = small.tile([128, QBLK], f32, tag="drc")
                          nc.vector.reciprocal(d_rc[:], ps_d[:])
                          nc.vector.tensor_mul(on[:, qs], ps_o[:], d_rc[:])
                  if n == N - 1:
                      on_last = on
                  else:
                      pending_ev.append((on, n))

            # ---------------- Phase C ----------------
            scope_c = nc.named_scope("C")
            scope_c.__enter__()
            flush_denoms()
            flush_evict()
            for qt in range(L // 128):
                oncat = wsl.tile([128, (N - 1) * 128], mdt, tag="oncat")
                nc.sync.dma_start(
                    out=oncat[:].rearrange("p (c j) -> p c j", c=N - 1),
                    in_=on_sc[0:(N - 1) * H, qt * 128:(qt + 1) * 128].rearrange(
                        "(c p) j -> p c j", p=128))
                on_tiles = [oncat[:, c * 128:(c + 1) * 128]
                            for c in range(N - 1)]
                on_tiles.append(on_last[:, qt * 128:(qt + 1) * 128])
                y_sb = evp.tile([128, E], f32, tag="ysb")
                ps_y = psA.tile([128, 1024], f32, tag="psA")
                for c in range(N):
                    for eb in range(E // 512):
                        nc.tensor.matmul(
                            ps_y[:, eb * 512:(eb + 1) * 512],
                            on_tiles[c],
                            pw_tiles[c][:, eb * 512:(eb + 1) * 512],
                            start=(c == 0), stop=(c == N - 1))
                nc.vector.tensor_add(y_sb[:], ps_y[:], pb_bc[:])
                nc.sync.dma_start(out=y_d[qt * 128:(qt + 1) * 128, :], in_=y_sb[:])
            scope_c.__exit__(None, None, None)

    nc.compile()
    return nc


def _get_program(mode=MODE):
    if mode not in _CACHE:
        _CACHE[mode] = _build(mode)
    return _CACHE[mode]


def kernel(k, q, w_kx, w_qx, proj_w, proj_b, mode=MODE):
    from concourse.bass_utils import run_bass_kernel_spmd

    k = np.asarray(k, dtype=np.float32)
    q = np.asarray(q, dtype=np.float32)
    w_kx = np.asarray(w_kx, dtype=np.float32)
    w_qx = np.asarray(w_qx, dtype=np.float32)
    proj_w = np.asarray(proj_w, dtype=np.float32)
    proj_b = np.asarray(proj_b, dtype=np.float32)

    rnd = _round_tf32 if mode == "f32r" else (
        lambda x: np.ascontiguousarray(x, dtype=np.float32))
    wk = rnd(w_kx.transpose(1, 0, 2).reshape(E, N * H))   # (e, n*h)
    wq = rnd(w_qx.transpose(1, 0, 2).reshape(E, N * H))
    pwT = rnd(proj_w.T)
    pb = np.ascontiguousarray(proj_b.reshape(1, E), dtype=np.float32)

    in_maps = []
    for b in range(NCORES):
        in_maps.append({
            "kT": rnd(k[b].T),
            "qT": rnd(q[b].T),
            "wk": wk,
            "wq": wq,
            "pwT": pwT,
            "pb": pb,
        })

    global _last_in_maps
    _last_in_maps = in_maps
    nc = _get_program(mode)
    res = run_bass_kernel_spmd(nc, in_maps, list(range(NCORES)))
    out = np.stack([res.results[b]["y"] for b in range(NCORES)], axis=0)
    return out.astype(np.float32)
